# revision 15
# baseline (speedup 1.0000x reference)
"""Trainium2 Bass kernel for LlamaApproximatedAttention (low-rank head-shared
scores + GQA values + o_proj).

Sharding: 8 cores = 4 batches x 2-way tensor-parallel over kv-heads.
Each core computes the full (causal-masked, head-shared) score/softmax for its
batch, the ctx for its half of the kv-heads, and a partial o_proj using the
GQA-folded Wo; the host sums the two partials per batch.

Math notes:
 - softmax is computed without max subtraction (scores are O(50) << 88, so
   exp cannot overflow fp32); normalization by the row sum is deferred and
   fused into the o_proj PSUM->SBUF copy (valid since probs are head-shared).
 - ctx for heads 2h and 2h+1 are identical (GQA repeat), so Wo is pre-folded
   on host: Wo_f[kv] = Wo[2kv] + Wo[2kv+1], halving the o_proj.
 - all matmuls run as float32r (full-rate fp32, ~1.5e-4 elementwise rel
   err); end-to-end relative error vs the fp32 reference is ~2.4e-3.

Measured (cost-model TimelineSim, per core): ~226 us end-to-end,
PE busy ~193 us (the kernel is tensor-engine bound, as intended).
"""
import sys
sys.path.insert(0, "/opt/trn_rl_repo")
import numpy as np

B, S, H = 4, 2048, 2048
RANK = 64
DHALF = 512          # kv-dim half handled per core (4 of 8 kv heads)
NEG = -1e9

_prog_cache = {}


def _build_program():
    import concourse.bacc as bacc
    import concourse.mybir as mybir
    import concourse.tile as tile

    f32 = mybir.dt.float32
    f32r = mybir.dt.float32r
    Exp = mybir.ActivationFunctionType.Exp
    Copy = mybir.ActivationFunctionType.Copy

    nc = bacc.Bacc("TRN2", target_bir_lowering=False, debug=False)
    hT = nc.declare_dram_parameter("hT", [H, S], f32r, isOutput=False)
    Wq = nc.declare_dram_parameter("Wq", [H, RANK], f32r, isOutput=False)
    Wk = nc.declare_dram_parameter("Wk", [H, RANK], f32r, isOutput=False)
    Wvh = nc.declare_dram_parameter("Wvh", [H, DHALF], f32r, isOutput=False)
    Wofh = nc.declare_dram_parameter("Wofh", [DHALF, H], f32r, isOutput=False)
    maskc = nc.declare_dram_parameter("maskc", [128, 896], f32, isOutput=False)
    ones = nc.declare_dram_parameter("ones", [128, 1], f32r, isOutput=False)
    outp = nc.declare_dram_parameter("outp", [S, H], f32, isOutput=True)

    with tile.TileContext(nc) as tc:
        with (
            tc.tile_pool(name="const", bufs=1) as constp,
            tc.tile_pool(name="wstream", bufs=1) as wsp,
            tc.tile_pool(name="hstream", bufs=2) as hsp,
            tc.tile_pool(name="persist", bufs=1) as pp,
            tc.tile_pool(name="probs", bufs=1) as probp,
            tc.tile_pool(name="work", bufs=2) as wkp,
            tc.tile_pool(name="outbuf", bufs=6) as obp,
            tc.tile_pool(name="dramp", bufs=2, space="DRAM") as drp,
            tc.tile_pool(name="ps", bufs=4, space="PSUM") as ps,
            tc.tile_pool(name="pskq", bufs=1, space="PSUM") as pskq,
            tc.tile_pool(name="pssums", bufs=2, space="PSUM") as pssums,
        ):
            # ---- constants / weights ----
            Wk_sb = constp.tile([128, 16, RANK], f32r, tag="wk")
            Wq_sb = constp.tile([128, 16, RANK], f32r, tag="wq")
            def load_w(dst, srcw, w0, w1):
                nc.sync.dma_start(
                    out=dst[:, w0:w1, :],
                    in_=srcw[w0 * 128:w1 * 128, :]
                        .rearrange("(t p) r -> p t r", p=128))
            load_w(Wk_sb, Wk, 0, 4)
            load_w(Wq_sb, Wq, 0, 4)
            # prefetch row-group 0 of hT before everything else big so the
            # first kq matmuls start as early as possible
            hT_pre = hsp.tile([128, 16, 256], f32r, tag="hT", name="hT_pre")
            for h0, h1 in ((0, 1), (1, 4), (4, 8), (8, 16)):
                nc.sync.dma_start(
                    out=hT_pre[:, h0:h1, :],
                    in_=hT[h0 * 128:h1 * 128, 0:256]
                        .rearrange("(t p) n -> p t n", p=128))
            for w0, w1 in ((4, 8), (8, 16)):
                load_w(Wk_sb, Wk, w0, w1)
                load_w(Wq_sb, Wq, w0, w1)
            Wvh_sb = wsp.tile([128, 16, DHALF], f32r, tag="wbig")
            for wc in range(4):
                nc.sync.dma_start(
                    out=Wvh_sb[:, 4 * wc:4 * (wc + 1), :],
                    in_=Wvh[wc * 512:(wc + 1) * 512, :]
                        .rearrange("(t p) d -> p t d", p=128))
            maskc_sb = constp.tile([128, 896], f32, tag="maskc")
            nc.sync.dma_start(out=maskc_sb[:], in_=maskc[:])
            ones_sb = constp.tile([128, 1], f32r, tag="ones")
            nc.sync.dma_start(out=ones_sb[:], in_=ones[:])

            kT_t = [pp.tile([64, 256], f32r, tag=f"kT{i}", name=f"kT{i}") for i in range(8)]
            qT_t = [pp.tile([64, 512], f32r, tag=f"qT{i}", name=f"qT{i}") for i in range(4)]
            v_sb = pp.tile([128, 16, DHALF], f32r, tag="v")

            def emit_scores(ib):
                njt = 4 * (ib + 1)
                probs = []
                sums_ps = pssums.tile([1, 512], f32, tag="sums",
                                      name=f"sums{ib}")
                for jt in range(njt):
                    sc_ps = ps.tile([128, 512], f32, tag="big",
                                    name=f"sc{ib}_{jt}")
                    # scoresT[j, i] = sum_r k[j,r] q[i,r]
                    nc.tensor.matmul(
                        sc_ps[:],
                        lhsT=kT_t[jt // 2][:, (jt % 2) * 128:(jt % 2 + 1) * 128],
                        rhs=qT_t[ib][:],
                        start=True, stop=True)
                    if jt >= 4 * ib:    # diagonal band: add causal mask
                        p = jt - 4 * ib
                        nc.vector.tensor_add(
                            sc_ps[:], sc_ps[:],
                            maskc_sb[:, 384 - 128 * p: 896 - 128 * p])
                    pt = probp.tile([128, 512], f32r, tag=f"p{jt}",
                                    name=f"p{ib}_{jt}")
                    nc.scalar.activation(pt[:], sc_ps[:], Exp)
                    probs.append(pt)
                    nc.tensor.matmul(sums_ps[:], lhsT=ones_sb[:],
                                     rhs=pt[:],
                                     start=(jt == 0), stop=(jt == njt - 1))
                return probs, sums_ps

            # ---- Phase A: projections (qT, kT over full batch; v half) ----
            for rg in range(8):            # row groups of 256
                if rg == 0:
                    hT_t = hT_pre
                else:
                    hT_t = hsp.tile([128, 16, 256], f32r, tag="hT")
                    for hc_ in range(4):
                        nc.sync.dma_start(
                            out=hT_t[:, 4 * hc_:4 * (hc_ + 1), :],
                            in_=hT[hc_ * 512:(hc_ + 1) * 512,
                                   rg * 256:(rg + 1) * 256]
                                .rearrange("(t p) n -> p t n", p=128))
                k_ps = pskq.tile([64, 256], f32, tag="kq0")
                q_ps = pskq.tile([64, 256], f32, tag="kq1")
                v_ps0 = ps.tile([128, DHALF], f32, tag="big")
                v_ps1 = ps.tile([128, DHALF], f32, tag="big")
                for h in range(16):
                    st, sp_ = (h == 0), (h == 15)
                    nc.tensor.matmul(k_ps[:], lhsT=Wk_sb[:, h, :],
                                     rhs=hT_t[:, h, :],
                                     start=st, stop=sp_)
                    nc.tensor.matmul(q_ps[:], lhsT=Wq_sb[:, h, :],
                                     rhs=hT_t[:, h, :],
                                     start=st, stop=sp_)
                for h in range(16):
                    st, sp_ = (h == 0), (h == 15)
                    nc.tensor.matmul(v_ps0[:],
                                     lhsT=hT_t[:, h, 0:128],
                                     rhs=Wvh_sb[:, h, :],
                                     start=st, stop=sp_)
                    nc.tensor.matmul(v_ps1[:],
                                     lhsT=hT_t[:, h, 128:256],
                                     rhs=Wvh_sb[:, h, :],
                                     start=st, stop=sp_)
                nc.vector.tensor_copy(kT_t[rg][:], k_ps[:])
                nc.vector.tensor_copy(
                    qT_t[rg // 2][:, (rg % 2) * 256:(rg % 2 + 1) * 256], q_ps[:])
                nc.vector.tensor_copy(v_sb[:, 2 * rg, :], v_ps0[:])
                nc.scalar.activation(v_sb[:, 2 * rg + 1, :], v_ps1[:], Copy)
                if rg == 1:
                    # ib0 scores/softmax only needs kT/qT of row-groups 0-1:
                    # hoist it here to fill phase A's DMA-bound stretches
                    hoisted = emit_scores(0)

            # o_proj weights reuse the Wvh slot (same tag) once phase A drains
            Wofh_sb = wsp.tile([128, 4, H], f32r, tag="wbig")
            for wc in range(4):
                nc.sync.dma_start(
                    out=Wofh_sb[:, wc:wc + 1, :],
                    in_=Wofh[wc * 128:(wc + 1) * 128, :]
                        .rearrange("(t p) n -> p t n", p=128))

            # ---- Phase B/C per 512-row i-block ----

            for ib in range(4):
                njt = 4 * (ib + 1)      # causal: j-tiles 0..njt-1
                i0 = ib * 512
                if ib == 0:
                    probs, sums_ps = hoisted
                else:
                    probs, sums_ps = emit_scores(ib)
                sums_sb = wkp.tile([1, 512], f32, tag="sums_sb")
                nc.vector.tensor_copy(sums_sb[:], sums_ps[:])
                # transpose [1,512] -> [128,4] via a DRAM bounce (DRAM APs are
                # linear, so re-partitioning the data there is legal)
                sums_dr = drp.tile([1, 512], f32, tag="sumsd")
                nc.sync.dma_start(out=sums_dr[:], in_=sums_sb[:])
                sumsT = wkp.tile([128, 4], f32, tag="sumsT")
                nc.sync.dma_start(
                    out=sumsT[:],
                    in_=sums_dr[:].rearrange("x (s p) -> p (x s)", p=128))
                recipT = wkp.tile([128, 4], f32, tag="recipT")
                nc.vector.reciprocal(recipT[:], sumsT[:])

                # ctxT[d, i] = sum_j v[j, d] * probsT[j, i]
                ctxs = []
                for dt_ in range(4):
                    cx_ps = ps.tile([128, 512], f32, tag="big")
                    for jt in range(njt):
                        nc.tensor.matmul(
                            cx_ps[:],
                            lhsT=v_sb[:, jt, dt_ * 128:(dt_ + 1) * 128]
                                ,
                            rhs=probs[jt][:],
                            start=(jt == 0), stop=(jt == njt - 1))
                    cx_sb = wkp.tile([128, 512], f32r, tag=f"ctx{dt_}")
                    nc.vector.tensor_copy(cx_sb[:], cx_ps[:])
                    ctxs.append(cx_sb)

                # o_proj partial: out[i, :] = (ctxT.T @ Wofh) / sums[i]
                for s_ in range(4):
                    for hc in range(4):
                        o_ps = ps.tile([128, 512], f32, tag="big")
                        for dt_ in range(4):
                            nc.tensor.matmul(
                                o_ps[:],
                                lhsT=ctxs[dt_][:, s_ * 128:(s_ + 1) * 128]
                                    ,
                                rhs=Wofh_sb[:, dt_, hc * 512:(hc + 1) * 512]
                                    ,
                                start=(dt_ == 0), stop=(dt_ == 3))
                        o_sb = obp.tile([128, 512], f32, tag="osb")
                        nc.scalar.activation(o_sb[:], o_ps[:], Copy,
                                             scale=recipT[:, s_:s_ + 1])
                        nc.sync.dma_start(
                            out=outp[i0 + s_ * 128: i0 + (s_ + 1) * 128,
                                     hc * 512:(hc + 1) * 512],
                            in_=o_sb[:])

    nc.compile()
    return nc


def _get_program():
    if "nc" not in _prog_cache:
        _prog_cache["nc"] = _build_program()
    return _prog_cache["nc"]


def _mask_is_causal(attention_mask):
    m = np.asarray(attention_mask)
    if m.shape != (B, 1, S, S):
        return False
    m0 = m[0, 0]
    iu = np.triu_indices(8, 1)
    # full check on batch 0, sampled check for batch equality
    tri_ok = (m0[np.tril_indices(S)] == 0.0).all()
    if not tri_ok:
        return False
    up = m0[np.triu_indices(S, 1)]
    if not (up <= -1e8).all():
        return False
    for b in range(1, B):
        if not np.array_equal(m[b, 0, ::257], m0[::257]):
            return False
    return True


def _fallback(hidden_states, attention_mask, Wq, Wk, Wv, Wo):
    hs = np.asarray(hidden_states, dtype=np.float64)
    q = hs @ np.asarray(Wq, np.float64)
    k = hs @ np.asarray(Wk, np.float64)
    v = (hs @ np.asarray(Wv, np.float64)).reshape(B, S, 8, 128)
    v = np.repeat(v.transpose(0, 2, 1, 3), 2, axis=1)
    scores = np.einsum("bir,bjr->bij", q, k) + np.asarray(
        attention_mask, np.float64)[:, 0]
    scores -= scores.max(axis=-1, keepdims=True)
    p = np.exp(scores)
    p /= p.sum(axis=-1, keepdims=True)
    ctx = np.einsum("bij,bhjd->bhid", p, v)
    ctx = ctx.transpose(0, 2, 1, 3).reshape(B, S, H)
    return (ctx @ np.asarray(Wo, np.float64)).astype(np.float32)


def kernel(hidden_states, attention_mask, Wq, Wk, Wv, Wo):
    hidden_states = np.ascontiguousarray(np.asarray(hidden_states, np.float32))
    Wq = np.ascontiguousarray(np.asarray(Wq, np.float32))
    Wk = np.ascontiguousarray(np.asarray(Wk, np.float32))
    Wv = np.ascontiguousarray(np.asarray(Wv, np.float32))
    Wo = np.ascontiguousarray(np.asarray(Wo, np.float32))

    if not _mask_is_causal(attention_mask):
        return _fallback(hidden_states, attention_mask, Wq, Wk, Wv, Wo)

    from concourse.bass_utils import run_bass_kernel_spmd

    nc = _get_program()

    # host-side prep
    Wof = Wo.reshape(8, 2, 128, H).sum(axis=1).reshape(1024, H)
    j = np.arange(128)[:, None]
    u = np.arange(896)[None, :]
    maskc = np.where(u >= j + 384, 0.0, NEG).astype(np.float32)
    ones = np.ones((128, 1), np.float32)
    hT = [np.ascontiguousarray(hidden_states[b].T) for b in range(B)]

    in_maps = []
    for c in range(8):
        b, dh = c // 2, c % 2
        in_maps.append({
            "hT": hT[b],
            "Wq": Wq, "Wk": Wk,
            "Wvh": np.ascontiguousarray(Wv[:, dh * DHALF:(dh + 1) * DHALF]),
            "Wofh": np.ascontiguousarray(Wof[dh * DHALF:(dh + 1) * DHALF, :]),
            "maskc": maskc, "ones": ones,
        })

    import os
    trace = bool(int(os.environ.get("KERNEL_TRACE", "0")))
    r = run_bass_kernel_spmd(nc, in_maps, core_ids=list(range(8)), trace=trace)
    _prog_cache["last_results"] = r

    out = np.empty((B, S, H), np.float32)
    for b in range(B):
        out[b] = r.results[2 * b]["outp"] + r.results[2 * b + 1]["outp"]
    return out


# revision 17
# speedup vs baseline: 1.0063x; 1.0063x over previous
"""Trainium2 Bass kernel for LlamaApproximatedAttention (low-rank head-shared
scores + GQA values + o_proj).

Sharding: 8 cores = 4 batches x 2-way tensor-parallel over kv-heads.
Each core computes the full (causal-masked, head-shared) score/softmax for its
batch, the ctx for its half of the kv-heads, and a partial o_proj using the
GQA-folded Wo; the host sums the two partials per batch.

Math notes:
 - softmax is computed without max subtraction (scores are O(50) << 88, so
   exp cannot overflow fp32); normalization by the row sum is deferred and
   fused into the o_proj PSUM->SBUF copy (valid since probs are head-shared).
 - ctx for heads 2h and 2h+1 are identical (GQA repeat), so Wo is pre-folded
   on host: Wo_f[kv] = Wo[2kv] + Wo[2kv+1], halving the o_proj.
 - all matmuls run as float32r (full-rate fp32, ~1.5e-4 elementwise rel
   err); end-to-end relative error vs the fp32 reference is ~2.4e-3.

Measured (cost-model TimelineSim, per core): ~226 us end-to-end,
PE busy ~193 us (the kernel is tensor-engine bound, as intended).
"""
import sys
sys.path.insert(0, "/opt/trn_rl_repo")
import numpy as np

B, S, H = 4, 2048, 2048
RANK = 64
DHALF = 512          # kv-dim half handled per core (4 of 8 kv heads)
NEG = -1e9

_prog_cache = {}


def _build_program():
    import concourse.bacc as bacc
    import concourse.mybir as mybir
    import concourse.tile as tile

    f32 = mybir.dt.float32
    f32r = mybir.dt.float32r
    Exp = mybir.ActivationFunctionType.Exp
    Copy = mybir.ActivationFunctionType.Copy

    nc = bacc.Bacc("TRN2", target_bir_lowering=False, debug=False)
    hT = nc.declare_dram_parameter("hT", [H, S], f32r, isOutput=False)
    Wq = nc.declare_dram_parameter("Wq", [H, RANK], f32r, isOutput=False)
    Wk = nc.declare_dram_parameter("Wk", [H, RANK], f32r, isOutput=False)
    Wvh = nc.declare_dram_parameter("Wvh", [H, DHALF], f32r, isOutput=False)
    Wofh = nc.declare_dram_parameter("Wofh", [DHALF, H], f32r, isOutput=False)
    maskc = nc.declare_dram_parameter("maskc", [128, 896], f32, isOutput=False)
    ones = nc.declare_dram_parameter("ones", [128, 1], f32r, isOutput=False)
    outp = nc.declare_dram_parameter("outp", [S, H], f32, isOutput=True)

    with tile.TileContext(nc) as tc:
        with (
            tc.tile_pool(name="const", bufs=1) as constp,
            tc.tile_pool(name="wstream", bufs=1) as wsp,
            tc.tile_pool(name="hstream", bufs=2) as hsp,
            tc.tile_pool(name="persist", bufs=1) as pp,
            tc.tile_pool(name="probs", bufs=1) as probp,
            tc.tile_pool(name="work", bufs=2) as wkp,
            tc.tile_pool(name="outbuf", bufs=6) as obp,
            tc.tile_pool(name="dramp", bufs=2, space="DRAM") as drp,
            tc.tile_pool(name="ps", bufs=4, space="PSUM") as ps,
            tc.tile_pool(name="pskq", bufs=1, space="PSUM") as pskq,
            tc.tile_pool(name="pssums", bufs=2, space="PSUM") as pssums,
        ):
            # ---- constants / weights ----
            Wk_sb = constp.tile([128, 16, RANK], f32r, tag="wk")
            Wq_sb = constp.tile([128, 16, RANK], f32r, tag="wq")
            def load_w(dst, srcw, w0, w1):
                nc.sync.dma_start(
                    out=dst[:, w0:w1, :],
                    in_=srcw[w0 * 128:w1 * 128, :]
                        .rearrange("(t p) r -> p t r", p=128))
            load_w(Wk_sb, Wk, 0, 4)
            load_w(Wq_sb, Wq, 0, 4)
            # prefetch row-group 0 of hT before everything else big so the
            # first kq matmuls start as early as possible
            hT_pre = hsp.tile([128, 16, 256], f32r, tag="hT", name="hT_pre")
            for h0, h1 in ((0, 1), (1, 4), (4, 8), (8, 16)):
                nc.sync.dma_start(
                    out=hT_pre[:, h0:h1, :],
                    in_=hT[h0 * 128:h1 * 128, 0:256]
                        .rearrange("(t p) n -> p t n", p=128))
            for w0, w1 in ((4, 8), (8, 16)):
                load_w(Wk_sb, Wk, w0, w1)
                load_w(Wq_sb, Wq, w0, w1)
            Wvh_sb = wsp.tile([128, 16, DHALF], f32r, tag="wbig")
            for wc in range(4):
                nc.sync.dma_start(
                    out=Wvh_sb[:, 4 * wc:4 * (wc + 1), :],
                    in_=Wvh[wc * 512:(wc + 1) * 512, :]
                        .rearrange("(t p) d -> p t d", p=128))
            maskc_sb = constp.tile([128, 896], f32, tag="maskc")
            nc.sync.dma_start(out=maskc_sb[:], in_=maskc[:])
            ones_sb = constp.tile([128, 1], f32r, tag="ones")
            nc.sync.dma_start(out=ones_sb[:], in_=ones[:])

            kT_t = [pp.tile([64, 256], f32r, tag=f"kT{i}", name=f"kT{i}") for i in range(8)]
            qT_t = [pp.tile([64, 512], f32r, tag=f"qT{i}", name=f"qT{i}") for i in range(4)]
            v_sb = pp.tile([128, 16, DHALF], f32r, tag="v")

            def emit_scores(ib, tag_off=0):
                njt = 4 * (ib + 1)
                probs = []
                sums_ps = pssums.tile([1, 512], f32, tag="sums",
                                      name=f"sums{ib}")
                for jt in range(njt):
                    sc_ps = ps.tile([128, 512], f32, tag="big",
                                    name=f"sc{ib}_{jt}")
                    # scoresT[j, i] = sum_r k[j,r] q[i,r]
                    nc.tensor.matmul(
                        sc_ps[:],
                        lhsT=kT_t[jt // 2][:, (jt % 2) * 128:(jt % 2 + 1) * 128],
                        rhs=qT_t[ib][:],
                        start=True, stop=True)
                    if jt >= 4 * ib:    # diagonal band: add causal mask
                        p = jt - 4 * ib
                        nc.vector.tensor_add(
                            sc_ps[:], sc_ps[:],
                            maskc_sb[:, 384 - 128 * p: 896 - 128 * p])
                    pt = probp.tile([128, 512], f32r,
                                    tag=f"p{(jt + tag_off) % 16}",
                                    name=f"p{ib}_{jt}")
                    nc.scalar.activation(pt[:], sc_ps[:], Exp)
                    probs.append(pt)
                    nc.tensor.matmul(sums_ps[:], lhsT=ones_sb[:],
                                     rhs=pt[:],
                                     start=(jt == 0), stop=(jt == njt - 1))
                return probs, sums_ps

            # ---- Phase A: projections (qT, kT over full batch; v half) ----
            for rg in range(8):            # row groups of 256
                if rg == 0:
                    hT_t = hT_pre
                else:
                    hT_t = hsp.tile([128, 16, 256], f32r, tag="hT")
                    for hc_ in range(4):
                        nc.sync.dma_start(
                            out=hT_t[:, 4 * hc_:4 * (hc_ + 1), :],
                            in_=hT[hc_ * 512:(hc_ + 1) * 512,
                                   rg * 256:(rg + 1) * 256]
                                .rearrange("(t p) n -> p t n", p=128))
                k_ps = pskq.tile([64, 256], f32, tag="kq0")
                q_ps = pskq.tile([64, 256], f32, tag="kq1")
                v_ps0 = ps.tile([128, DHALF], f32, tag="big")
                v_ps1 = ps.tile([128, DHALF], f32, tag="big")
                for h in range(16):
                    st, sp_ = (h == 0), (h == 15)
                    nc.tensor.matmul(k_ps[:], lhsT=Wk_sb[:, h, :],
                                     rhs=hT_t[:, h, :],
                                     start=st, stop=sp_)
                    nc.tensor.matmul(q_ps[:], lhsT=Wq_sb[:, h, :],
                                     rhs=hT_t[:, h, :],
                                     start=st, stop=sp_)
                for h in range(16):
                    st, sp_ = (h == 0), (h == 15)
                    nc.tensor.matmul(v_ps0[:],
                                     lhsT=hT_t[:, h, 0:128],
                                     rhs=Wvh_sb[:, h, :],
                                     start=st, stop=sp_)
                    nc.tensor.matmul(v_ps1[:],
                                     lhsT=hT_t[:, h, 128:256],
                                     rhs=Wvh_sb[:, h, :],
                                     start=st, stop=sp_)
                nc.vector.tensor_copy(kT_t[rg][:], k_ps[:])
                nc.vector.tensor_copy(
                    qT_t[rg // 2][:, (rg % 2) * 256:(rg % 2 + 1) * 256], q_ps[:])
                nc.vector.tensor_copy(v_sb[:, 2 * rg, :], v_ps0[:])
                nc.scalar.activation(v_sb[:, 2 * rg + 1, :], v_ps1[:], Copy)
                if rg == 1:
                    # ib0 scores/softmax only needs kT/qT of row-groups 0-1:
                    # hoist it here to fill phase A's DMA-bound stretches
                    hoisted = {0: emit_scores(0)}
                if rg == 3:
                    # same for ib1 (needs row-groups 0-3)
                    hoisted[1] = emit_scores(1, tag_off=8)

            # o_proj weights reuse the Wvh slot (same tag) once phase A drains
            Wofh_sb = wsp.tile([128, 4, H], f32r, tag="wbig")
            for wc in range(4):
                nc.sync.dma_start(
                    out=Wofh_sb[:, wc:wc + 1, :],
                    in_=Wofh[wc * 128:(wc + 1) * 128, :]
                        .rearrange("(t p) n -> p t n", p=128))

            # ---- Phase B/C per 512-row i-block ----

            for ib in range(4):
                njt = 4 * (ib + 1)      # causal: j-tiles 0..njt-1
                i0 = ib * 512
                if ib in hoisted:
                    probs, sums_ps = hoisted[ib]
                else:
                    probs, sums_ps = emit_scores(ib)
                sums_sb = wkp.tile([1, 512], f32, tag="sums_sb")
                nc.vector.tensor_copy(sums_sb[:], sums_ps[:])
                # transpose [1,512] -> [128,4] via a DRAM bounce (DRAM APs are
                # linear, so re-partitioning the data there is legal)
                sums_dr = drp.tile([1, 512], f32, tag="sumsd")
                nc.sync.dma_start(out=sums_dr[:], in_=sums_sb[:])
                sumsT = wkp.tile([128, 4], f32, tag="sumsT")
                nc.sync.dma_start(
                    out=sumsT[:],
                    in_=sums_dr[:].rearrange("x (s p) -> p (x s)", p=128))
                recipT = wkp.tile([128, 4], f32, tag="recipT")
                nc.vector.reciprocal(recipT[:], sumsT[:])

                # ctxT[d, i] = sum_j v[j, d] * probsT[j, i]
                ctxs = []
                for dt_ in range(4):
                    cx_ps = ps.tile([128, 512], f32, tag="big")
                    for jt in range(njt):
                        nc.tensor.matmul(
                            cx_ps[:],
                            lhsT=v_sb[:, jt, dt_ * 128:(dt_ + 1) * 128]
                                ,
                            rhs=probs[jt][:],
                            start=(jt == 0), stop=(jt == njt - 1))
                    cx_sb = wkp.tile([128, 512], f32r, tag=f"ctx{dt_}")
                    nc.vector.tensor_copy(cx_sb[:], cx_ps[:])
                    ctxs.append(cx_sb)

                # o_proj partial: out[i, :] = (ctxT.T @ Wofh) / sums[i]
                for s_ in range(4):
                    for hc in range(4):
                        o_ps = ps.tile([128, 512], f32, tag="big")
                        for dt_ in range(4):
                            nc.tensor.matmul(
                                o_ps[:],
                                lhsT=ctxs[dt_][:, s_ * 128:(s_ + 1) * 128]
                                    ,
                                rhs=Wofh_sb[:, dt_, hc * 512:(hc + 1) * 512]
                                    ,
                                start=(dt_ == 0), stop=(dt_ == 3))
                        o_sb = obp.tile([128, 512], f32, tag="osb")
                        nc.scalar.activation(o_sb[:], o_ps[:], Copy,
                                             scale=recipT[:, s_:s_ + 1])
                        nc.sync.dma_start(
                            out=outp[i0 + s_ * 128: i0 + (s_ + 1) * 128,
                                     hc * 512:(hc + 1) * 512],
                            in_=o_sb[:])

    nc.compile()
    return nc


def _get_program():
    if "nc" not in _prog_cache:
        _prog_cache["nc"] = _build_program()
    return _prog_cache["nc"]


def _mask_is_causal(attention_mask):
    m = np.asarray(attention_mask)
    if m.shape != (B, 1, S, S):
        return False
    m0 = m[0, 0]
    iu = np.triu_indices(8, 1)
    # full check on batch 0, sampled check for batch equality
    tri_ok = (m0[np.tril_indices(S)] == 0.0).all()
    if not tri_ok:
        return False
    up = m0[np.triu_indices(S, 1)]
    if not (up <= -1e8).all():
        return False
    for b in range(1, B):
        if not np.array_equal(m[b, 0, ::257], m0[::257]):
            return False
    return True


def _fallback(hidden_states, attention_mask, Wq, Wk, Wv, Wo):
    hs = np.asarray(hidden_states, dtype=np.float64)
    q = hs @ np.asarray(Wq, np.float64)
    k = hs @ np.asarray(Wk, np.float64)
    v = (hs @ np.asarray(Wv, np.float64)).reshape(B, S, 8, 128)
    v = np.repeat(v.transpose(0, 2, 1, 3), 2, axis=1)
    scores = np.einsum("bir,bjr->bij", q, k) + np.asarray(
        attention_mask, np.float64)[:, 0]
    scores -= scores.max(axis=-1, keepdims=True)
    p = np.exp(scores)
    p /= p.sum(axis=-1, keepdims=True)
    ctx = np.einsum("bij,bhjd->bhid", p, v)
    ctx = ctx.transpose(0, 2, 1, 3).reshape(B, S, H)
    return (ctx @ np.asarray(Wo, np.float64)).astype(np.float32)


def kernel(hidden_states, attention_mask, Wq, Wk, Wv, Wo):
    hidden_states = np.ascontiguousarray(np.asarray(hidden_states, np.float32))
    Wq = np.ascontiguousarray(np.asarray(Wq, np.float32))
    Wk = np.ascontiguousarray(np.asarray(Wk, np.float32))
    Wv = np.ascontiguousarray(np.asarray(Wv, np.float32))
    Wo = np.ascontiguousarray(np.asarray(Wo, np.float32))

    if not _mask_is_causal(attention_mask):
        return _fallback(hidden_states, attention_mask, Wq, Wk, Wv, Wo)

    from concourse.bass_utils import run_bass_kernel_spmd

    nc = _get_program()

    # host-side prep
    Wof = Wo.reshape(8, 2, 128, H).sum(axis=1).reshape(1024, H)
    j = np.arange(128)[:, None]
    u = np.arange(896)[None, :]
    maskc = np.where(u >= j + 384, 0.0, NEG).astype(np.float32)
    ones = np.ones((128, 1), np.float32)
    hT = [np.ascontiguousarray(hidden_states[b].T) for b in range(B)]

    in_maps = []
    for c in range(8):
        b, dh = c // 2, c % 2
        in_maps.append({
            "hT": hT[b],
            "Wq": Wq, "Wk": Wk,
            "Wvh": np.ascontiguousarray(Wv[:, dh * DHALF:(dh + 1) * DHALF]),
            "Wofh": np.ascontiguousarray(Wof[dh * DHALF:(dh + 1) * DHALF, :]),
            "maskc": maskc, "ones": ones,
        })

    import os
    trace = bool(int(os.environ.get("KERNEL_TRACE", "0")))
    r = run_bass_kernel_spmd(nc, in_maps, core_ids=list(range(8)), trace=trace)
    _prog_cache["last_results"] = r

    out = np.empty((B, S, H), np.float32)
    for b in range(B):
        out[b] = r.results[2 * b]["outp"] + r.results[2 * b + 1]["outp"]
    return out


# revision 18
# speedup vs baseline: 1.0559x; 1.0493x over previous
"""Trainium2 Bass kernel for LlamaApproximatedAttention (low-rank head-shared
scores + GQA values + o_proj).

Sharding: 8 cores = 4 batches x 2-way tensor-parallel over kv-heads.
Each core computes the full (causal-masked, head-shared) score/softmax for its
batch, the ctx for its half of the kv-heads, and a partial o_proj using the
GQA-folded Wo; the host sums the two partials per batch.

Math notes:
 - softmax is computed without max subtraction (scores are O(50) << 88, so
   exp cannot overflow fp32); normalization by the row sum is deferred and
   fused into the o_proj PSUM->SBUF copy (valid since probs are head-shared).
 - ctx for heads 2h and 2h+1 are identical (GQA repeat), so Wo is pre-folded
   on host: Wo_f[kv] = Wo[2kv] + Wo[2kv+1], halving the o_proj.
 - all matmuls run as float32r (full-rate fp32, ~1.5e-4 elementwise rel
   err); end-to-end relative error vs the fp32 reference is ~2.4e-3.

Measured (cost-model TimelineSim, per core): ~226 us end-to-end,
PE busy ~193 us (the kernel is tensor-engine bound, as intended).
"""
import sys
sys.path.insert(0, "/opt/trn_rl_repo")
import numpy as np

B, S, H = 4, 2048, 2048
RANK = 64
DHALF = 512          # kv-dim half handled per core (4 of 8 kv heads)
NEG = -1e9

_prog_cache = {}


def _build_program():
    import concourse.bacc as bacc
    import concourse.mybir as mybir
    import concourse.tile as tile

    f32 = mybir.dt.float32
    f32r = mybir.dt.float32r
    Exp = mybir.ActivationFunctionType.Exp
    Copy = mybir.ActivationFunctionType.Copy

    nc = bacc.Bacc("TRN2", target_bir_lowering=False, debug=False)
    hT = nc.declare_dram_parameter("hT", [H, S], f32r, isOutput=False)
    Wq = nc.declare_dram_parameter("Wq", [H, RANK], f32r, isOutput=False)
    Wk = nc.declare_dram_parameter("Wk", [H, RANK], f32r, isOutput=False)
    Wvh = nc.declare_dram_parameter("Wvh", [H, DHALF], f32r, isOutput=False)
    Wofh = nc.declare_dram_parameter("Wofh", [DHALF, H], f32r, isOutput=False)
    maskc = nc.declare_dram_parameter("maskc", [128, 896], f32, isOutput=False)
    ones = nc.declare_dram_parameter("ones", [128, 1], f32r, isOutput=False)
    outp = nc.declare_dram_parameter("outp", [S, H], f32, isOutput=True)

    with tile.TileContext(nc) as tc:
        with (
            tc.tile_pool(name="const", bufs=1) as constp,
            tc.tile_pool(name="wstream", bufs=1) as wsp,
            tc.tile_pool(name="hstream", bufs=2) as hsp,
            tc.tile_pool(name="persist", bufs=1) as pp,
            tc.tile_pool(name="probs", bufs=1) as probp,
            tc.tile_pool(name="work", bufs=2) as wkp,
            tc.tile_pool(name="outbuf", bufs=6) as obp,
            tc.tile_pool(name="dramp", bufs=2, space="DRAM") as drp,
            tc.tile_pool(name="ps", bufs=4, space="PSUM") as ps,
            tc.tile_pool(name="pskq", bufs=1, space="PSUM") as pskq,
            tc.tile_pool(name="pssums", bufs=2, space="PSUM") as pssums,
        ):
            # ---- constants / weights ----
            Wk_sb = constp.tile([128, 16, RANK], f32r, tag="wk")
            Wq_sb = constp.tile([128, 16, RANK], f32r, tag="wq")
            def load_w(dst, srcw, w0, w1):
                nc.sync.dma_start(
                    out=dst[:, w0:w1, :],
                    in_=srcw[w0 * 128:w1 * 128, :]
                        .rearrange("(t p) r -> p t r", p=128))
            load_w(Wk_sb, Wk, 0, 4)
            load_w(Wq_sb, Wq, 0, 4)
            # prefetch row-group 0 of hT before everything else big so the
            # first kq matmuls start as early as possible
            hT_pre = hsp.tile([128, 16, 256], f32r, tag="hT", name="hT_pre")
            for h0, h1 in ((0, 1), (1, 4), (4, 8), (8, 16)):
                nc.sync.dma_start(
                    out=hT_pre[:, h0:h1, :],
                    in_=hT[h0 * 128:h1 * 128, 0:256]
                        .rearrange("(t p) n -> p t n", p=128))
            for w0, w1 in ((4, 8), (8, 16)):
                load_w(Wk_sb, Wk, w0, w1)
                load_w(Wq_sb, Wq, w0, w1)
            Wvh_sb = wsp.tile([128, 16, DHALF], f32r, tag="wbig")
            for wc in range(4):
                nc.sync.dma_start(
                    out=Wvh_sb[:, 4 * wc:4 * (wc + 1), :],
                    in_=Wvh[wc * 512:(wc + 1) * 512, :]
                        .rearrange("(t p) d -> p t d", p=128))
            maskc_sb = constp.tile([128, 896], f32, tag="maskc")
            nc.sync.dma_start(out=maskc_sb[:], in_=maskc[:])
            ones_sb = constp.tile([128, 1], f32r, tag="ones")
            nc.sync.dma_start(out=ones_sb[:], in_=ones[:])

            kT_t = [pp.tile([64, 256], f32r, tag=f"kT{i}", name=f"kT{i}") for i in range(8)]
            qT_t = [pp.tile([64, 512], f32r, tag=f"qT{i}", name=f"qT{i}") for i in range(4)]
            v_sb = pp.tile([128, 16, DHALF], f32r, tag="v")

            def emit_scores(ib, tag_off=0):
                njt = 4 * (ib + 1)
                probs = []
                sums_ps = pssums.tile([1, 512], f32, tag="sums",
                                      name=f"sums{ib}")
                for jt in range(njt):
                    sc_ps = ps.tile([128, 512], f32, tag="big",
                                    name=f"sc{ib}_{jt}")
                    # scoresT[j, i] = sum_r k[j,r] q[i,r]
                    nc.tensor.matmul(
                        sc_ps[:],
                        lhsT=kT_t[jt // 2][:, (jt % 2) * 128:(jt % 2 + 1) * 128],
                        rhs=qT_t[ib][:],
                        start=True, stop=True)
                    if jt >= 4 * ib:    # diagonal band: add causal mask
                        p = jt - 4 * ib
                        nc.vector.tensor_add(
                            sc_ps[:], sc_ps[:],
                            maskc_sb[:, 384 - 128 * p: 896 - 128 * p])
                    pt = probp.tile([128, 512], f32r,
                                    tag=f"p{(jt + tag_off) % 16}",
                                    name=f"p{ib}_{jt}")
                    nc.scalar.activation(pt[:], sc_ps[:], Exp)
                    probs.append(pt)
                    nc.tensor.matmul(sums_ps[:], lhsT=ones_sb[:],
                                     rhs=pt[:],
                                     start=(jt == 0), stop=(jt == njt - 1))
                return probs, sums_ps

            # ---- Phase A: projections (qT, kT over full batch; v half) ----
            for rg in range(8):            # row groups of 256
                if rg == 0:
                    hT_t = hT_pre
                else:
                    hT_t = hsp.tile([128, 16, 256], f32r, tag="hT")
                    for hc_ in range(4):
                        nc.sync.dma_start(
                            out=hT_t[:, 4 * hc_:4 * (hc_ + 1), :],
                            in_=hT[hc_ * 512:(hc_ + 1) * 512,
                                   rg * 256:(rg + 1) * 256]
                                .rearrange("(t p) n -> p t n", p=128))
                k_ps = pskq.tile([64, 256], f32, tag="kq0")
                q_ps = pskq.tile([64, 256], f32, tag="kq1")
                v_ps0 = ps.tile([128, DHALF], f32, tag="big")
                v_ps1 = ps.tile([128, DHALF], f32, tag="big")
                for h in range(16):
                    st, sp_ = (h == 0), (h == 15)
                    nc.tensor.matmul(k_ps[:], lhsT=Wk_sb[:, h, :],
                                     rhs=hT_t[:, h, :],
                                     start=st, stop=sp_)
                    nc.tensor.matmul(q_ps[:], lhsT=Wq_sb[:, h, :],
                                     rhs=hT_t[:, h, :],
                                     start=st, stop=sp_)
                for h in range(16):
                    st, sp_ = (h == 0), (h == 15)
                    nc.tensor.matmul(v_ps0[:],
                                     lhsT=hT_t[:, h, 0:128],
                                     rhs=Wvh_sb[:, h, :],
                                     start=st, stop=sp_)
                    nc.tensor.matmul(v_ps1[:],
                                     lhsT=hT_t[:, h, 128:256],
                                     rhs=Wvh_sb[:, h, :],
                                     start=st, stop=sp_)
                nc.vector.tensor_copy(kT_t[rg][:], k_ps[:])
                nc.vector.tensor_copy(
                    qT_t[rg // 2][:, (rg % 2) * 256:(rg % 2 + 1) * 256], q_ps[:])
                nc.vector.tensor_copy(v_sb[:, 2 * rg, :], v_ps0[:])
                nc.scalar.activation(v_sb[:, 2 * rg + 1, :], v_ps1[:], Copy)
                if rg == 1:
                    # ib0 scores/softmax only needs kT/qT of row-groups 0-1:
                    # hoist it here to fill phase A's DMA-bound stretches
                    hoisted = {0: emit_scores(0)}
                if rg == 3:
                    # same for ib1 (needs row-groups 0-3)
                    hoisted[1] = emit_scores(1, tag_off=8)

            # o_proj weights reuse the Wvh slot (same tag) once phase A drains
            Wofh_sb = wsp.tile([128, 4, H], f32r, tag="wbig")
            for wc in range(4):
                nc.sync.dma_start(
                    out=Wofh_sb[:, wc:wc + 1, :],
                    in_=Wofh[wc * 128:(wc + 1) * 128, :]
                        .rearrange("(t p) n -> p t n", p=128))

            # ---- Phase B/C per 512-row i-block ----

            for ib in range(4):
                njt = 4 * (ib + 1)      # causal: j-tiles 0..njt-1
                i0 = ib * 512
                if ib in hoisted:
                    probs, sums_ps = hoisted[ib]
                else:
                    probs, sums_ps = emit_scores(ib)
                sums_sb = wkp.tile([1, 512], f32, tag="sums_sb")
                nc.vector.tensor_copy(sums_sb[:], sums_ps[:])
                # transpose [1,512] -> [128,4] via a DRAM bounce (DRAM APs are
                # linear, so re-partitioning the data there is legal)
                sums_dr = drp.tile([1, 512], f32, tag="sumsd")
                nc.sync.dma_start(out=sums_dr[:], in_=sums_sb[:])
                sumsT = wkp.tile([128, 4], f32, tag="sumsT")
                nc.sync.dma_start(
                    out=sumsT[:],
                    in_=sums_dr[:].rearrange("x (s p) -> p (x s)", p=128))
                recipT = wkp.tile([128, 4], f32, tag="recipT")
                nc.vector.reciprocal(recipT[:], sumsT[:])

                # ctxT[d, i] = sum_j v[j, d] * probsT[j, i]
                ctxs = []
                for dt_ in range(4):
                    # reuse the kq PSUM banks (idle after phase A) so the
                    # "big" slots stay free for scores/o-proj rotation
                    cx_ps = pskq.tile([128, 512], f32, tag=("kq0" if dt_ % 2 == 0 else "kq1"),
                                      name=f"cx{ib}_{dt_}")
                    for jt in range(njt):
                        nc.tensor.matmul(
                            cx_ps[:],
                            lhsT=v_sb[:, jt, dt_ * 128:(dt_ + 1) * 128]
                                ,
                            rhs=probs[jt][:],
                            start=(jt == 0), stop=(jt == njt - 1))
                    cx_sb = wkp.tile([128, 512], f32r, tag=f"ctx{dt_}")
                    nc.vector.tensor_copy(cx_sb[:], cx_ps[:])
                    ctxs.append(cx_sb)

                # o_proj partial: out[i, :] = (ctxT.T @ Wofh) / sums[i]
                for s_ in range(4):
                    for hc in range(4):
                        o_ps = ps.tile([128, 512], f32, tag="big")
                        for dt_ in range(4):
                            nc.tensor.matmul(
                                o_ps[:],
                                lhsT=ctxs[dt_][:, s_ * 128:(s_ + 1) * 128]
                                    ,
                                rhs=Wofh_sb[:, dt_, hc * 512:(hc + 1) * 512]
                                    ,
                                start=(dt_ == 0), stop=(dt_ == 3))
                        o_sb = obp.tile([128, 512], f32, tag="osb")
                        nc.scalar.activation(o_sb[:], o_ps[:], Copy,
                                             scale=recipT[:, s_:s_ + 1])
                        nc.sync.dma_start(
                            out=outp[i0 + s_ * 128: i0 + (s_ + 1) * 128,
                                     hc * 512:(hc + 1) * 512],
                            in_=o_sb[:])

    nc.compile()
    return nc


def _get_program():
    if "nc" not in _prog_cache:
        _prog_cache["nc"] = _build_program()
    return _prog_cache["nc"]


def _mask_is_causal(attention_mask):
    m = np.asarray(attention_mask)
    if m.shape != (B, 1, S, S):
        return False
    m0 = m[0, 0]
    iu = np.triu_indices(8, 1)
    # full check on batch 0, sampled check for batch equality
    tri_ok = (m0[np.tril_indices(S)] == 0.0).all()
    if not tri_ok:
        return False
    up = m0[np.triu_indices(S, 1)]
    if not (up <= -1e8).all():
        return False
    for b in range(1, B):
        if not np.array_equal(m[b, 0, ::257], m0[::257]):
            return False
    return True


def _fallback(hidden_states, attention_mask, Wq, Wk, Wv, Wo):
    hs = np.asarray(hidden_states, dtype=np.float64)
    q = hs @ np.asarray(Wq, np.float64)
    k = hs @ np.asarray(Wk, np.float64)
    v = (hs @ np.asarray(Wv, np.float64)).reshape(B, S, 8, 128)
    v = np.repeat(v.transpose(0, 2, 1, 3), 2, axis=1)
    scores = np.einsum("bir,bjr->bij", q, k) + np.asarray(
        attention_mask, np.float64)[:, 0]
    scores -= scores.max(axis=-1, keepdims=True)
    p = np.exp(scores)
    p /= p.sum(axis=-1, keepdims=True)
    ctx = np.einsum("bij,bhjd->bhid", p, v)
    ctx = ctx.transpose(0, 2, 1, 3).reshape(B, S, H)
    return (ctx @ np.asarray(Wo, np.float64)).astype(np.float32)


def kernel(hidden_states, attention_mask, Wq, Wk, Wv, Wo):
    hidden_states = np.ascontiguousarray(np.asarray(hidden_states, np.float32))
    Wq = np.ascontiguousarray(np.asarray(Wq, np.float32))
    Wk = np.ascontiguousarray(np.asarray(Wk, np.float32))
    Wv = np.ascontiguousarray(np.asarray(Wv, np.float32))
    Wo = np.ascontiguousarray(np.asarray(Wo, np.float32))

    if not _mask_is_causal(attention_mask):
        return _fallback(hidden_states, attention_mask, Wq, Wk, Wv, Wo)

    from concourse.bass_utils import run_bass_kernel_spmd

    nc = _get_program()

    # host-side prep
    Wof = Wo.reshape(8, 2, 128, H).sum(axis=1).reshape(1024, H)
    j = np.arange(128)[:, None]
    u = np.arange(896)[None, :]
    maskc = np.where(u >= j + 384, 0.0, NEG).astype(np.float32)
    ones = np.ones((128, 1), np.float32)
    hT = [np.ascontiguousarray(hidden_states[b].T) for b in range(B)]

    in_maps = []
    for c in range(8):
        b, dh = c // 2, c % 2
        in_maps.append({
            "hT": hT[b],
            "Wq": Wq, "Wk": Wk,
            "Wvh": np.ascontiguousarray(Wv[:, dh * DHALF:(dh + 1) * DHALF]),
            "Wofh": np.ascontiguousarray(Wof[dh * DHALF:(dh + 1) * DHALF, :]),
            "maskc": maskc, "ones": ones,
        })

    import os
    trace = bool(int(os.environ.get("KERNEL_TRACE", "0")))
    r = run_bass_kernel_spmd(nc, in_maps, core_ids=list(range(8)), trace=trace)
    _prog_cache["last_results"] = r

    out = np.empty((B, S, H), np.float32)
    for b in range(B):
        out[b] = r.results[2 * b]["outp"] + r.results[2 * b + 1]["outp"]
    return out
